# revision 10
# baseline (speedup 1.0000x reference)
"""TRN2 Bass/Tile kernel: windowed additive (Bahdanau) attention + gated combine.

Per (batch b, time t):
  win      = enc[t-16 : t+17]                        (zero padded, 33-wide)
  energy   = tanh(x[t] @ Wh + win @ We + b_attn)     [33, 256]
  scores   = energy @ v                              [33]
  a        = softmax(scores)
  weighted = a @ win                                 [256]
  out      = sigmoid(concat(x[t], weighted) @ W_lin + b_lin)
Returns (out [B,T,256], a [B,T,1,33]).

Sharding: sequence-parallel over T across 8 cores (T/8 = 256 positions per
batch per core, x4 batches = 1024 positions/core), 16-wide zero halo on
encoder_outputs.

On-chip strategy (per core, per batch-block of 256 positions):
  * feature-on-partition layout: hxT = Wh.T @ x.T  [256o, 256t],
    heT = WeA.T @ [encT; ones] [256o, 288c] (b_attn folded via ones row,
    so zero-padded halo columns get exactly +b_attn like the reference).
  * energy: ONE fused DVE add per 128-feature half using a broadcast AP for
    hx ([0,33] step) and an overlapping sliding-window AP for he
    ([1,33],[1,256] steps), out [128, 33*256] bf16; ONE big ACT tanh over it.
  * scores, directly transposed: per (half, offset d, t-half) a PE matmul
    with lhsT = E slice [128o, 128t] and rhs = one-hot-expanded V [128, 33]
    (column d = v), PSUM-accumulated into scoresT [128t, 33] — softmax layout,
    no evac/transpose needed.
  * softmax per 128-position sub-block straight off PSUM: DVE -max,
    ACT exp(bias=-max, accum_out=sum), DVE reciprocal + per-partition mul.
  * weighted = banded a @ enc: a rows are written into a zero-padded DRAM
    scratch and read back with row stride (PADW-1), which shears the band
    into a dense matrix; PE-transpose gives S[c, t] and weightedT comes from
    2 PSUM-accumulated matmuls against natural-layout enc tiles.
  * final: outT = sigmoid(W_lin.T @ [xT; weightedT] + b_lin) via 6 fp32r
    matmuls per half + ACT sigmoid-with-bias, stored transposed (host
    untransposes).
  * inputs are host-packed so each block needs few, large DMAs; loads issue
    on the sync queue while data-dependent stores/scratch go on the gpsimd
    queue to avoid head-of-line blocking.
"""

import os
import numpy as np

B, T, MODEL, OUTF = 4, 2048, 512, 256
W_SIZE, WIN = 16, 33
NCORES = 8
TC = T // NCORES            # 256 positions per batch per core
M = TC + 2 * W_SIZE         # 288 enc columns per block (16 halo each side)
NB = B                      # blocks per core (one per batch)
SUB = 128                   # sub-block for softmax / banded matmul
PADW = 287                  # scratch row width (zero padded)
POFF = 127                  # a-window column offset inside a scratch row

last_exec_time_ns = None
_cache = {}


def _build_nc():
    import concourse.bacc as bacc
    import concourse.tile as tile
    from concourse import mybir, masks
    from concourse.ap import AP

    f32 = mybir.dt.float32
    f32r = mybir.dt.float32r
    bf16 = mybir.dt.bfloat16
    TANH = mybir.ActivationFunctionType.Tanh
    EXP = mybir.ActivationFunctionType.Exp
    SIG = mybir.ActivationFunctionType.Sigmoid
    ADD = mybir.AluOpType.add
    MAXOP = mybir.AluOpType.max
    AXX = mybir.AxisListType.X

    nc = bacc.Bacc("TRN2", target_bir_lowering=False, debug=False,
                   num_devices=NCORES)

    # host-packed inputs (one DMA per tile group)
    xP = nc.dram_tensor("xP", [128, 4, NB, TC], f32r, kind="ExternalInput")
    eP = nc.dram_tensor("eP", [128, 2, NB, M], f32r, kind="ExternalInput")
    onesP = nc.dram_tensor("onesP", [1, NB, M], f32r, kind="ExternalInput")
    enP = nc.dram_tensor("enP", [128, 3, NB, OUTF], f32r, kind="ExternalInput")
    WhP = nc.dram_tensor("WhP", [128, 4 * 2 * 128], f32r, kind="ExternalInput")
    WeP = nc.dram_tensor("WeP", [128, 2 * 2 * 128], f32r, kind="ExternalInput")
    We2P = nc.dram_tensor("We2P", [1, 2 * 128], f32r, kind="ExternalInput")
    WlP = nc.dram_tensor("WlP", [128, 6 * 2 * 128], f32r, kind="ExternalInput")
    VP = nc.dram_tensor("VP", [128, 2 * WIN * WIN], f32, kind="ExternalInput")
    blP = nc.dram_tensor("blP", [128, 2], f32, kind="ExternalInput")
    outT = nc.dram_tensor("outT", [OUTF, NB * TC], f32, kind="ExternalOutput")
    aout = nc.dram_tensor("aout", [NB * TC, WIN], f32, kind="ExternalOutput")

    with tile.TileContext(nc) as tc:
        with (
            tc.tile_pool(name="pw", bufs=1) as pw,
            tc.tile_pool(name="pio", bufs=2) as pio,
            tc.tile_pool(name="pE", bufs=2) as pE,
            tc.tile_pool(name="pdram", bufs=1, space="DRAM") as pdram,
            tc.tile_pool(name="pps", bufs=2, space="PSUM") as pps,
        ):
            whT = pw.tile([128, 4 * 2 * 128], f32r, tag="whT")
            nc.sync.dma_start(whT[:], WhP.ap())
            weT = pw.tile([128, 2 * 2 * 128], f32r, tag="weT")
            nc.sync.dma_start(weT[:], WeP.ap())
            we2T = pw.tile([1, 2 * 128], f32r, tag="we2T")
            nc.sync.dma_start(we2T[:], We2P.ap())
            wlT = pw.tile([128, 6 * 2 * 128], f32r, tag="wlT")
            nc.sync.dma_start(wlT[:], WlP.ap())
            vfT = pw.tile([128, 2 * WIN * WIN], f32, tag="vfT")
            nc.sync.dma_start(vfT[:], VP.ap())
            vbT = pw.tile([128, 2 * WIN * WIN], bf16, tag="vbT")
            nc.vector.tensor_copy(vbT[:], vfT[:])
            blT = pw.tile([128, 2], f32, tag="blT")
            nc.sync.dma_start(blT[:], blP.ap())

            def wh(k, m):
                return whT[:, (k * 2 + m) * 128:(k * 2 + m + 1) * 128]

            def weA(k, m):
                return weT[:, (k * 2 + m) * 128:(k * 2 + m + 1) * 128]

            def wl(k, m):
                return wlT[:, (k * 2 + m) * 128:(k * 2 + m + 1) * 128]

            def Vb(h, d):
                return vbT[:, h * WIN * WIN + d * WIN:
                           h * WIN * WIN + (d + 1) * WIN]

            idf = pw.tile([128, 128], f32, tag="idf")
            masks.make_identity(nc, idf[:])

            # DRAM scratch for the band shear: 2 slots, zeroed once. Only the
            # a-window columns [POFF, POFF+WIN) are ever rewritten, so the
            # zero padding persists across reuses.
            zt = pw.tile([128, PADW], f32, tag="zt")
            nc.gpsimd.memset(zt[:], 0.0)
            scratch = pdram.tile([2, SUB, PADW], f32, tag="scr")
            nc.gpsimd.dma_start(scratch[0], zt[:])
            nc.gpsimd.dma_start(scratch[1], zt[:])

            for b in range(NB):
                xk = pio.tile([128, 4, TC], f32r, tag="xk")
                nc.sync.dma_start(xk[:], xP.ap()[:, :, b, :])
                ek = pio.tile([128, 2, M], f32r, tag="ek")
                nc.sync.dma_start(ek[:], eP.ap()[:, :, b, :])
                ones = pio.tile([1, M], f32r, tag="ones")
                nc.sync.dma_start(ones[:], onesP.ap()[:, b, :])
                en = pio.tile([128, 3, OUTF], f32r, tag="en")
                nc.sync.dma_start(en[:], enP.ap()[:, :, b, :])

                # hxT = Wh.T @ x.T   [2x128 o, 256 t] -> bf16
                hxb = []
                for mh in range(2):
                    ps = pps.tile([128, TC], f32, tag="hxhe", bufs=2,
                                  name=f"hxps{mh}_{b}")
                    for k in range(4):
                        nc.tensor.matmul(ps[:], wh(k, mh), xk[:, k, :],
                                         start=(k == 0), stop=(k == 3))
                    t = pio.tile([128, TC], bf16, tag=f"hx{mh}",
                                 name=f"hx{mh}_{b}")
                    nc.vector.tensor_copy(t[:], ps[:])
                    hxb.append(t)

                # heT = WeA.T @ [encT; ones]  [2x128 o, 288 c] -> bf16
                heb = []
                for mh in range(2):
                    ps = pps.tile([128, M], f32, tag="hxhe", bufs=2,
                                  name=f"heps{mh}_{b}")
                    for k in range(2):
                        nc.tensor.matmul(ps[:], weA(k, mh), ek[:, k, :],
                                         start=(k == 0), stop=False)
                    nc.tensor.matmul(ps[:], we2T[:, mh * 128:(mh + 1) * 128],
                                     ones[:], start=False, stop=True)
                    t = pio.tile([128, M], bf16, tag=f"he{mh}",
                                 name=f"he{mh}_{b}")
                    nc.vector.tensor_copy(t[:], ps[:])
                    heb.append(t)

                # energy (fused add) + tanh + transposed score matmuls
                scT = []
                for th in range(2):
                    scT.append(pps.tile([128, WIN], f32, tag="scT",
                                        bufs=2, name=f"scT{th}_{b}"))
                for h in range(2):
                    E = pE.tile([128, WIN * TC], bf16, tag=f"E{h}",
                                name=f"E{h}_{b}")
                    Ev = E[:].rearrange("p (d t) -> p d t", d=WIN)
                    hxv = hxb[h][:].unsqueeze(1).broadcast_to([128, WIN, TC])
                    hea = heb[h][:]
                    hewin = AP(hea.tensor, hea.offset,
                               [list(hea.ap[0]), [1, WIN], [1, TC]],
                               dep_tracking_offset=hea.dep_tracking_offset)
                    nc.vector.tensor_tensor(Ev, hxv, hewin, ADD)
                    nc.scalar.activation(E[:], E[:], TANH)
                    for d in range(WIN):
                        for th in range(2):
                            nc.tensor.matmul(
                                scT[th][:],
                                E[:, d * TC + th * SUB:d * TC + th * SUB + SUB],
                                Vb(h, d),
                                start=(h == 0 and d == 0),
                                stop=(h == 1 and d == WIN - 1))

                wsb = []
                for mh in range(2):
                    wt = pio.tile([128, TC], f32r, tag=f"wsb{mh}",
                                  name=f"wsb{mh}_{b}")
                    wsb.append(wt)

                for s in range(2):
                    tl0 = s * SUB
                    negmax = pio.tile([128, 1], f32, tag="negmax",
                                      name=f"negmax_{b}_{s}")
                    nc.vector.tensor_reduce(negmax[:], scT[s][:], axis=AXX,
                                            op=MAXOP, negate=True)
                    expv = pio.tile([128, WIN], f32, tag="expv",
                                    name=f"expv_{b}_{s}")
                    sume = pio.tile([128, 1], f32, tag="sume",
                                    name=f"sume_{b}_{s}")
                    nc.scalar.activation(expv[:], scT[s][:], EXP,
                                         bias=negmax[:], accum_out=sume[:])
                    rec = pio.tile([128, 1], f32, tag="rec",
                                   name=f"rec_{b}_{s}")
                    nc.vector.reciprocal(rec[:], sume[:])
                    af = pio.tile([128, WIN], f32, tag="af",
                                  name=f"af_{b}_{s}")
                    nc.vector.tensor_scalar_mul(af[:], expv[:], rec[:])
                    row0 = b * TC + tl0
                    nc.gpsimd.dma_start(aout.ap()[row0:row0 + SUB, :], af[:])

                    # band shear via DRAM scratch
                    sl = (2 * b + s) % 2
                    nc.gpsimd.dma_start(scratch[sl][:, POFF:POFF + WIN], af[:])
                    ash = pio.tile([128, 160], f32, tag="ash",
                                   name=f"ash_{b}_{s}")
                    bsl = scratch[sl][:, POFF:POFF + 160]
                    skew = AP(bsl.tensor, bsl.offset,
                              [[PADW - 1, SUB], [1, 160]],
                              dep_tracking_offset=bsl.dep_tracking_offset)
                    nc.gpsimd.dma_start(ash[:], skew)
                    # S = shear(a).T : Shi [128,128] cols 0:128, Slo [32,128]
                    # cols 128:256 of one packed PSUM tile
                    Sp = pps.tile([128, 256], f32, tag="swps", bufs=2,
                                  name=f"Sp_{b}_{s}")
                    nc.tensor.transpose(Sp[:, 0:128], ash[:, 0:128], idf[:])
                    nc.tensor.transpose(Sp[0:32, 128:256], ash[:, 128:160],
                                        idf[:])
                    shs = pio.tile([128, SUB], f32r, tag="shs",
                                   name=f"shs_{b}_{s}")
                    sls = pio.tile([32, SUB], f32r, tag="sls",
                                   name=f"sls_{b}_{s}")
                    nc.vector.tensor_copy(shs[:], Sp[:, 0:128])
                    nc.vector.tensor_copy(sls[:], Sp[0:32, 128:256])

                    # weightedT[f, tl] = enc_band.T @ S  (PSUM-accumulated),
                    # both f halves packed into one PSUM tile
                    wp = pps.tile([128, 256], f32, tag="swps", bufs=2,
                                  name=f"wp_{b}_{s}")
                    for mh in range(2):
                        lo = en[:, s, :][:, mh * 128:(mh + 1) * 128]
                        hi = en[0:32, s + 1, :][:, mh * 128:(mh + 1) * 128]
                        nc.tensor.matmul(wp[:, mh * 128:(mh + 1) * 128], lo,
                                         shs[:], start=True, stop=False)
                        nc.tensor.matmul(wp[:, mh * 128:(mh + 1) * 128], hi,
                                         sls[:], start=False, stop=True)
                    for mh in range(2):
                        nc.vector.tensor_copy(
                            wsb[mh][:, tl0:tl0 + SUB],
                            wp[:, mh * 128:(mh + 1) * 128])

                # final: outT = sigmoid(Wl.T @ [xT; weightedT] + b_lin),
                # both halves packed into one PSUM tile + one output DMA
                po = pps.tile([128, 2 * TC], f32, tag="ops", bufs=2,
                              name=f"po_{b}")
                osb = pio.tile([128, 2, TC], f32, tag="osb", name=f"osb_{b}")
                for mh in range(2):
                    sl_ = po[:, mh * TC:(mh + 1) * TC]
                    for k in range(4):
                        nc.tensor.matmul(sl_, wl(k, mh), xk[:, k, :],
                                         start=(k == 0), stop=False)
                    for k2 in range(2):
                        nc.tensor.matmul(sl_, wl(4 + k2, mh), wsb[k2][:],
                                         start=False, stop=(k2 == 1))
                    nc.scalar.activation(osb[:, mh, :], sl_, SIG,
                                         bias=blT[:, mh:mh + 1])
                ot = outT.ap().rearrange("(m p) t -> p m t", m=2)
                nc.gpsimd.dma_start(ot[:, :, b * TC:(b + 1) * TC], osb[:])

    nc.compile()
    return nc


def _get_nc():
    if "nc" not in _cache:
        _cache["nc"] = _build_nc()
    return _cache["nc"]


def _prep_in_maps(x, encoder_outputs, W_attn, b_attn, v, W_lin, b_lin):
    x = np.ascontiguousarray(np.asarray(x, dtype=np.float32))
    enc = np.ascontiguousarray(np.asarray(encoder_outputs, dtype=np.float32))
    W_attn = np.asarray(W_attn, dtype=np.float32)
    b_attn = np.asarray(b_attn, dtype=np.float32)
    v = np.asarray(v, dtype=np.float32)
    W_lin = np.ascontiguousarray(np.asarray(W_lin, dtype=np.float32))
    b_lin = np.asarray(b_lin, dtype=np.float32)

    encp = np.pad(enc, ((0, 0), (W_SIZE, W_SIZE), (0, 0)))

    def pack_w(Wmat, nk):  # [nk*128, 256] -> [128, nk*2*128]
        t = Wmat.reshape(nk, 128, 2, 128)
        return np.ascontiguousarray(t.transpose(1, 0, 2, 3).reshape(128, -1))

    WhP_ = pack_w(W_attn[:MODEL], 4)
    We_ = W_attn[MODEL:]
    WeP_ = pack_w(We_, 2)
    We2P_ = np.ascontiguousarray(b_attn.reshape(1, 2 * 128))
    WlP_ = pack_w(W_lin, 6)
    VP_ = np.zeros((128, 2 * WIN * WIN), np.float32)
    for h in range(2):
        for d in range(WIN):
            VP_[:, h * WIN * WIN + d * WIN + d] = v[h * 128:(h + 1) * 128]
    VP_ = np.ascontiguousarray(VP_)
    blP_ = np.ascontiguousarray(b_lin.reshape(2, 128).T)

    in_maps = []
    for c in range(NCORES):
        xs = x[:, c * TC:(c + 1) * TC, :]            # [B, 256, 512]
        # xP[r, k, b, t] = x[b, t, k*128+r]
        xP_ = np.ascontiguousarray(xs.transpose(2, 0, 1).reshape(
            4, 128, NB, TC).transpose(1, 0, 2, 3))
        es = encp[:, c * TC:c * TC + M, :]           # [B, 288, 256]
        eT = es.transpose(2, 0, 1)                   # [256, B, 288]
        eP_ = np.ascontiguousarray(eT.reshape(2, 128, NB, M).transpose(1, 0, 2, 3))
        onesP_ = np.ascontiguousarray(np.ones((1, NB, M), np.float32))
        # enP[r, g, b, f] = es[b, g*128+r, f]  (g=2 rows 32.. are zero pad)
        enP_ = np.zeros((128, 3, NB, OUTF), np.float32)
        esn = es.transpose(1, 0, 2)                  # [288, B, 256]
        enP_[:, 0] = esn[0:128]
        enP_[:, 1] = esn[128:256]
        enP_[0:32, 2] = esn[256:288]
        enP_ = np.ascontiguousarray(enP_)
        in_maps.append({
            "xP": xP_, "eP": eP_, "onesP": onesP_, "enP": enP_,
            "WhP": WhP_, "WeP": WeP_, "We2P": We2P_, "WlP": WlP_,
            "VP": VP_, "blP": blP_,
        })
    return in_maps


def kernel(x, encoder_outputs, W_attn, b_attn, v, W_lin, b_lin):
    global last_exec_time_ns
    from concourse.bass_utils import run_bass_kernel_spmd

    nc = _get_nc()
    in_maps = _prep_in_maps(x, encoder_outputs, W_attn, b_attn, v, W_lin,
                            b_lin)
    trace = os.environ.get("KERNEL_TRACE") == "1"
    res = run_bass_kernel_spmd(nc, in_maps, core_ids=list(range(NCORES)),
                               trace=trace)
    last_exec_time_ns = res.exec_time_ns

    out = np.empty((B, T, OUTF), dtype=np.float32)
    a = np.empty((B, T, WIN), dtype=np.float32)
    for c in range(NCORES):
        r = res.results[c]
        out[:, c * TC:(c + 1) * TC, :] = (
            r["outT"].reshape(OUTF, NB, TC).transpose(1, 2, 0))
        a[:, c * TC:(c + 1) * TC, :] = r["aout"].reshape(NB, TC, WIN)
    return out, a[:, :, None, :]
